# revision 47
# baseline (speedup 1.0000x reference)
"""MLA (multi-head latent attention) TRN2 kernel.

Sharding: 8 cores = 2 batches x 4 head-groups (4 heads each).
Each core computes a partial [2048, 2048] output (its 4 heads through
out_proj); host sums the 4 partials per batch.

Device layout notes:
- All "T" tensors are stored transposed: [feature_dim (partition), seq (free)].
- q/k rope dims are split even/odd on host so rope is pure elementwise.
- rms_norm weight and softmax scale are absorbed into weights on host.
- walrus allows only ONE semaphore wait per instruction, so the code is
  structured so every instruction needs at most one fresh cross-engine
  frontier: inputs load via single consolidated DMAs, and tiny "touch"
  ops (1x1 matmul into a dedicated PSUM bank / 1-elem copies) pre-acquire
  frontiers before join instructions.
"""

import numpy as np

import bass_rust

import concourse.bass as bass
import concourse.mybir as mybir
from concourse.tile import TileContext
from concourse.vector_clock import VectorClock, ScopedClock
from concourse.bass_utils import run_bass_kernel_spmd

F32 = mybir.dt.float32
AF = mybir.ActivationFunctionType
OP = mybir.AluOpType

B = 2
S = 2048
D = 2048
H_PER = 4          # heads per core
KVR = 512          # kv lora rank
ROPE = 64
EPS = 1e-6
MASKV = -30000.0
NT = 4             # seq tiles of 512
W = 256            # half-tile width for projections
USE_F32R = True


class ChunkedDrainTileContext(TileContext):
    # walrus here rejects >1 sync-wait per instruction; pre-wait the global
    # clock via single-wait SP nops, then drain with no waits (SP is in-order).
    def _drain_and_barrier(self, tick_clock, wait_clock):
        gc = tick_clock.global_clock
        n = len(gc)
        vals = [gc[i] for i in range(n)]
        for i in range(n):
            if vals[i] <= 0:
                continue
            partial = VectorClock([vals[j] if j == i else 0 for j in range(n)])
            nop = self.nc.sync.nop(nofuse=True, hint=f"drain_wait_{i}")
            wait_clock.add_sem_waits(nop.ins, ScopedClock({None: partial}))
        self.nc.sync.drain()
        self.nc.all_engine_barrier()
        assert self.sems is not None
        popped = self.nc._tile_sem_poison_stack.pop()
        assert popped is self._sem_poison
        self.nc.clear_and_free_semaphores(list(self.sems.allocated().values()))
        self.nc.all_engine_barrier()


def build_nc(use_f32r=USE_F32R):
    nc = bass.Bass()
    # MD: dtype for every matmul operand. float32r = fp32 PE fast mode
    # (1 cyc/row vs 4); producers must write it natively so walrus sees
    # rounded data.
    MD = mybir.dt.float32r if use_f32r else F32
    xT = nc.declare_dram_parameter("xT", [128, 16, S], MD, isOutput=False)
    wq = nc.declare_dram_parameter("wq", [128, 16, 512], MD, isOutput=False)
    wkva = nc.declare_dram_parameter("wkva", [128, 16, 576], MD, isOutput=False)
    wkbk = nc.declare_dram_parameter("wkbk", [128, 4, 256], MD, isOutput=False)
    wkbv = nc.declare_dram_parameter("wkbv", [128, 4, 512], MD, isOutput=False)
    wout = nc.declare_dram_parameter("wout", [128, 4, D], MD, isOutput=False)
    cossin = nc.declare_dram_parameter("cossin", [128, 2, S], F32, isOutput=False)
    # f32r constants must be DMAed (Memset can't write f32r):
    # col 0 = ones_col, cols 1:129 = repE rows 0:64, cols 129:257 = repO
    constsd = nc.declare_dram_parameter("consts", [128, 257], MD, isOutput=False)
    outp = nc.declare_dram_parameter("outp", [S, D], F32, isOutput=True)

    def mm(out, lhsT, rhs, start, stop, tile_position=None):
        nc.tensor.matmul(out, lhsT, rhs, start=start, stop=stop,
                         tile_position=tile_position)

    # attention outputs spilled per-tile; phase D reads them back.
    # index [:, 4*t+h, :] = head h of q-tile t
    avd = nc.dram_tensor("avd", [128, 16, 512], MD, kind="Internal")

    with ChunkedDrainTileContext(nc) as tc:
        with tc.tile_pool(name="persist", bufs=1) as pp:
            kn = pp.tile([128, 2 * S], MD)        # pair p -> cols p*S
            # k_pe rope tables duplicated over the 4 32-row blocks so scores
            # matmuls see matching base partitions for every head
            kpeE = pp.tile([128, S], MD)
            kpeO = pp.tile([128, S], MD)
            vs = pp.tile([128, 16 * 512], MD)     # key-block m -> cols m*512
            masks = pp.tile([128, 896], F32)      # diag j -> cols (3-j)*128
            consts_sb = pp.tile([128, 257], MD)
            ones_row = pp.tile([1, 128], F32)     # K=1 mms stay plain fp32
            eps_c = pp.tile([1, 1], F32)
            ones_col = consts_sb[:, 0:1]
            repE = consts_sb[0:64, 1:129]         # repE[c,m] = (m%32 == c)
            repO = consts_sb[0:64, 129:257]       # repO[c,m] = (m%32 == c-32)

            nc.sync.dma_start(consts_sb[:], constsd[:, :])
            nc.gpsimd.memset(ones_row[:], 1.0)
            nc.gpsimd.memset(eps_c[:], EPS)
            # keep 0 where (col - part - 384) >= 0 else MASKV; diag block j
            # reads cols (3-j)*128 .. +512
            nc.gpsimd.memset(masks[:], 0.0)
            nc.gpsimd.affine_select(
                out=masks[:], in_=masks[:], compare_op=OP.is_ge, fill=MASKV,
                base=-384, channel_multiplier=-1, pattern=[[1, 896]])

            if True:
                with tc.tile_pool(name="main", bufs=1) as mp:
                    wq_sb = mp.tile([128, 16 * 512], MD)
                    wkva_sb = mp.tile([128, 16 * 576], MD)
                    wkbk_sb = mp.tile([128, 4 * 256], MD)
                    wkbv_sb = mp.tile([128, 4 * 512], MD)
                    qt = mp.tile([128, 4 * 512], MD)   # N0|N1|E|O per q-tile
                    x_sb = mp.tile([128, 16 * W], MD)
                    ck = mp.tile([128, 4 * W], MD)

                    nc.sync.dma_start(wq_sb[:], wq[:, :, :])
                    nc.sync.dma_start(wkva_sb[:], wkva[:, :, :])
                    nc.sync.dma_start(wkbk_sb[:], wkbk[:, :, :])
                    nc.sync.dma_start(wkbv_sb[:], wkbv[:, :, :])

                    # warm-ups: acquire each static frontier one at a time

                    with tc.tile_pool(name="tp", bufs=2) as tp:
                        for t in range(NT):
                            for half in range(2):
                                st = 2 * t + half
                                off = st * W
                                nc.sync.dma_start(x_sb[:], xT[:, :, off:off + W])
                                cs2 = tp.tile([128, 2 * W], F32, tag="cossin", bufs=1)
                                nc.sync.dma_start(cs2[:], cossin[:, :, off:off + W])
                                cos_sb = cs2[:, 0:W]
                                sin_sb = cs2[:, W:2 * W]

                                with tc.tile_pool(name="psA", bufs=1, space="PSUM") as psA:
                                    ckv_ps = [psA.tile([128, W], F32, name=f"ckv{i}")
                                              for i in range(4)]
                                    # rows 0:64 hold raw k_pe; after eviction the
                                    # full bank is reused for the replicated-E copy
                                    kpe_ps = psA.tile([128, W], F32, name="kpe_ps")
                                    kO_ps = psA.tile([128, W], F32, name="kO_ps")
                                    for k in range(16):
                                        xk = x_sb[:, k * W:(k + 1) * W]
                                        for i in range(4):
                                            mm(ckv_ps[i][:],
                                               wkva_sb[:, k * 576 + i * 128:k * 576 + (i + 1) * 128],
                                               xk, start=(k == 0), stop=(k == 15))
                                        mm(kpe_ps[0:64, :],
                                           wkva_sb[:, k * 576 + 512:k * 576 + 576],
                                           xk, start=(k == 0), stop=(k == 15))
                                    ssq_ps = psA.tile([1, W], F32, name="ssq_ps")
                                    for i in range(4):
                                        nc.vector.tensor_copy(
                                            out=ck[:, i * W:(i + 1) * W], in_=ckv_ps[i][:])
                                        sq = tp.tile([128, W], MD, tag="sq")
                                        nc.scalar.activation(sq[:], ckv_ps[i][:], AF.Square)
                                        mm(ssq_ps[:], ones_col, sq[:],
                                           start=(i == 0), stop=(i == 3))
                                    rst = tp.tile([1, W], F32, tag="rst")
                                    nc.scalar.activation(rst[:], ssq_ps[:], AF.Sqrt,
                                                         scale=1.0 / KVR, bias=eps_c[:])
                                    rstr = tp.tile([1, W], F32, tag="rstr", bufs=1)
                                    nc.vector.reciprocal(rstr[:], rst[:])
                                    # k_pe rope: evict raw, replicate 4x over
                                    # partitions via rep matmuls, then rotate
                                    kraw = tp.tile([64, W], MD, tag="kraw", bufs=1)
                                    nc.scalar.copy(kraw[:], kpe_ps[0:64, :])
                                    mm(kpe_ps[:], repE, kraw[:], start=True, stop=True)
                                    mm(kO_ps[:], repO, kraw[:], start=True, stop=True)
                                    kE = kpeE[:, off:off + W]
                                    kO = kpeO[:, off:off + W]
                                    ta = tp.tile([128, W], F32, tag="ta", bufs=1)
                                    tb = tp.tile([128, W], F32, tag="tb", bufs=1)
                                    nc.vector.tensor_tensor(out=kE, in0=kpe_ps[:], in1=cos_sb, op=OP.mult)
                                    nc.vector.tensor_tensor(out=ta[:], in0=kO_ps[:], in1=sin_sb, op=OP.mult)
                                    nc.vector.tensor_tensor(out=kE, in0=kE, in1=ta[:], op=OP.subtract)
                                    nc.vector.tensor_tensor(out=kO, in0=kO_ps[:], in1=cos_sb, op=OP.mult)
                                    nc.vector.tensor_tensor(out=tb[:], in0=kpe_ps[:], in1=sin_sb, op=OP.mult)
                                    nc.vector.tensor_tensor(out=kO, in0=kO, in1=tb[:], op=OP.add)

                                with tc.tile_pool(name="psB", bufs=1, space="PSUM") as psB:
                                    bc_ps = psB.tile([128, W], F32, name="bc_ps")
                                    mm(bc_ps[:], ones_row[:], rstr[:], start=True, stop=True)
                                    for i in range(4):
                                        ci = ck[:, i * W:(i + 1) * W]
                                        nc.vector.tensor_tensor(out=ci, in0=ci, in1=bc_ps[:], op=OP.mult)

                                with tc.tile_pool(name="psC", bufs=1, space="PSUM") as psC:
                                    # kn accumulates in cols 0:W, then the full
                                    # bank is reused for the vs accumulation
                                    knv = [psC.tile([128, 512], F32, name=f"knv{p}")
                                           for p in range(2)]
                                    for p in range(2):
                                        for cc in range(4):
                                            mm(knv[p][:, 0:W],
                                               wkbk_sb[:, cc * 256 + p * 128:cc * 256 + (p + 1) * 128],
                                               ck[:, cc * W:(cc + 1) * W],
                                               start=(cc == 0), stop=(cc == 3))
                                        nc.scalar.copy(kn[:, p * S + off:p * S + off + W],
                                                       knv[p][:, 0:W])
                                    for u in range(2):
                                        for cc in range(4):
                                            mm(knv[u][:],
                                               ck[:, cc * W + u * 128:cc * W + u * 128 + 128],
                                               wkbv_sb[:, cc * 512:(cc + 1) * 512],
                                               start=(cc == 0), stop=(cc == 3))
                                        nc.vector.tensor_copy(
                                            out=vs[:, (2 * st + u) * 512:(2 * st + u + 1) * 512],
                                            in_=knv[u][:])
                                    qps = [psC.tile([128, W], F32, name=f"q{j}")
                                           for j in range(4)]
                                    for k in range(16):
                                        xk = x_sb[:, k * W:(k + 1) * W]
                                        for j in range(4):
                                            mm(qps[j][:],
                                               wq_sb[:, k * 512 + j * 128:k * 512 + (j + 1) * 128],
                                               xk, start=(k == 0), stop=(k == 15))
                                    for j in range(2):
                                        nc.scalar.copy(
                                            qt[:, j * 512 + half * W:j * 512 + half * W + W],
                                            qps[j][:])
                                    qE = qt[:, 1024 + half * W:1024 + half * W + W]
                                    qO = qt[:, 1536 + half * W:1536 + half * W + W]
                                    tq1 = tp.tile([128, W], F32, tag="tq1", bufs=1)
                                    tq2 = tp.tile([128, W], F32, tag="tq2", bufs=1)
                                    nc.vector.tensor_tensor(out=qE, in0=qps[2][:], in1=cos_sb, op=OP.mult)
                                    nc.vector.tensor_tensor(out=tq1[:], in0=qps[3][:], in1=sin_sb, op=OP.mult)
                                    nc.vector.tensor_tensor(out=qE, in0=qE, in1=tq1[:], op=OP.subtract)
                                    nc.vector.tensor_tensor(out=qO, in0=qps[3][:], in1=cos_sb, op=OP.mult)
                                    nc.vector.tensor_tensor(out=tq2[:], in0=qps[2][:], in1=sin_sb, op=OP.mult)
                                    nc.vector.tensor_tensor(out=qO, in0=qO, in1=tq2[:], op=OP.add)

                            # attention for q-tile t
                            with tc.tile_pool(name="psT", bufs=1, space="PSUM") as psT:
                                av4 = tp.tile([128, 4 * 512], MD, tag="av4", bufs=1)
                                nb = 4 * t + 4
                                for h in range(H_PER):
                                    p = h // 2
                                    o = (h % 2) * 64
                                    cs_ps = psT.tile([1, 512], F32, tag="cs")
                                    av_ps = psT.tile([128, 512], F32, tag="av")
                                    for kb in range(nb):
                                        s_ps = psT.tile([128, 512], F32, tag=f"s{kb % 2}")
                                        mm(s_ps[:],
                                           kn[o:o + 64, p * S + kb * 128:p * S + (kb + 1) * 128],
                                           qt[o:o + 64, p * 512:(p + 1) * 512],
                                           start=True, stop=False)
                                        mm(s_ps[:],
                                           kpeE[32 * h:32 * h + 32, kb * 128:(kb + 1) * 128],
                                           qt[32 * h:32 * h + 32, 1024:1536],
                                           start=False, stop=False,
                                           tile_position=(32 * h, 0))
                                        mm(s_ps[:],
                                           kpeO[32 * h:32 * h + 32, kb * 128:(kb + 1) * 128],
                                           qt[32 * h:32 * h + 32, 1536:2048],
                                           start=False, stop=True,
                                           tile_position=(32 * h, 0))
                                        E = tp.tile([128, 512], MD, tag="E")
                                        j = kb - 4 * t
                                        if j >= 0:
                                            nc.vector.tensor_tensor(
                                                out=E[:], in0=s_ps[:],
                                                in1=masks[:, (3 - j) * 128:(3 - j) * 128 + 512],
                                                op=OP.add)
                                            nc.scalar.activation(E[:], E[:], AF.Exp)
                                        else:
                                            nc.scalar.activation(E[:], s_ps[:], AF.Exp)
                                        mm(cs_ps[:], ones_col, E[:],
                                           start=(kb == 0), stop=(kb == nb - 1))
                                        mm(av_ps[:],
                                           vs[:, kb * 512 + h * 128:kb * 512 + h * 128 + 128],
                                           E[:], start=(kb == 0), stop=(kb == nb - 1))
                                    cs_sb = tp.tile([1, 512], F32, tag="cssb", bufs=1)
                                    nc.scalar.copy(cs_sb[:], cs_ps[:])
                                    rc_sb = tp.tile([1, 512], F32, tag="rcsb", bufs=1)
                                    nc.vector.reciprocal(rc_sb[:], cs_sb[:])
                                    rb_ps = psT.tile([128, 512], F32, tag="rb")
                                    mm(rb_ps[:], ones_row[:], rc_sb[:], start=True, stop=True)
                                    rbs = tp.tile([128, 512], F32, tag="rbs", bufs=1)
                                    nc.scalar.copy(rbs[:], rb_ps[:])
                                    nc.vector.tensor_tensor(
                                        out=av4[:, h * 512:(h + 1) * 512],
                                        in0=av_ps[:], in1=rbs[:], op=OP.mult)
                                nc.sync.dma_start(avd[:, 4 * t:4 * t + 4, :], av4[:])

    # phase D: out = sum_h av_h.T @ wout_h (av read back from DRAM).
    # Separate tile context: the drain barrier between contexts resets all
    # frontiers, so phase D starts clean.
    with ChunkedDrainTileContext(nc) as tc2:
        with tc2.tile_pool(name="dp", bufs=1) as dp:
            wout_sb = dp.tile([128, 4 * D], MD)
            nc.sync.dma_start(wout_sb[:], wout[:, :, :])
            with tc2.tile_pool(name="dstage", bufs=2) as dsp:
                with tc2.tile_pool(name="psD", bufs=1, space="PSUM") as psD:
                    for m in range(16):
                        t, r = m // 4, m % 4
                        avm = dsp.tile([128, 4 * 128], MD, tag="avm")
                        nc.sync.dma_start(
                            avm[:], avd[:, 4 * t:4 * t + 4, r * 128:(r + 1) * 128])
                        stage = dsp.tile([128, D], F32, tag="stage")
                        for n in range(4):
                            d_ps = psD.tile([128, 512], F32, tag=f"d{n % 2}")
                            for hh in range(4):
                                mm(d_ps[:],
                                   avm[:, hh * 128:(hh + 1) * 128],
                                   wout_sb[:, hh * D + n * 512:hh * D + (n + 1) * 512],
                                   start=(hh == 0), stop=(hh == 3))
                            nc.scalar.copy(stage[:, n * 512:(n + 1) * 512], d_ps[:])
                        nc.sync.dma_start(outp[m * 128:(m + 1) * 128, :], stage[:])
    # walrus allows at most 1 sem wait per instruction; this Bacc pass
    # splits multi-wait instructions (the spmd path doesn't run it)
    bass_rust.generate_event_semaphores(nc)
    return nc


def _prep_inputs(x, w_query, wkv_a, wkv_b, kv_norm_w, out_proj_w):
    scale = 128.0 ** -0.5
    f = np.float32
    ins = []
    wkvb_n = (wkv_b * kv_norm_w[:, None]).astype(f)
    freqs = 1.0 / (10000.0 ** (np.arange(0, ROPE, 2, dtype=np.float64) / ROPE))
    ang = np.outer(freqs, np.arange(S, dtype=np.float64))
    cosd = np.tile(np.cos(ang), (4, 1)).astype(f)
    sind = np.tile(np.sin(ang), (4, 1)).astype(f)
    cossin = np.ascontiguousarray(np.stack([cosd, sind], axis=1))  # [128,2,S]
    consts = np.zeros((128, 257), f)
    consts[:, 0] = 1.0
    m = np.arange(128)
    for cc in range(32):
        consts[cc, 1 + np.flatnonzero(m % 32 == cc)] = 1.0        # repE
        consts[32 + cc, 129 + np.flatnonzero(m % 32 == cc)] = 1.0  # repO

    def to_pkc(a, nk):  # [128*nk, c] -> [128, nk, c]
        return np.ascontiguousarray(
            a.reshape(nk, 128, a.shape[1]).transpose(1, 0, 2))

    for c in range(8):
        b, hg = c // 4, c % 4
        heads = [4 * hg + l for l in range(4)]
        cols = []
        for l in (0, 1):
            cols.append(w_query[:, 128 * heads[l]:128 * heads[l] + 64])
        for l in (2, 3):
            cols.append(w_query[:, 128 * heads[l]:128 * heads[l] + 64])
        for l in range(4):
            cols.append(w_query[:, 128 * heads[l] + 64:128 * heads[l] + 128:2])
        for l in range(4):
            cols.append(w_query[:, 128 * heads[l] + 65:128 * heads[l] + 128:2])
        wq_p = (np.concatenate(cols, axis=1) * scale).astype(f)
        wq_p = wq_p.reshape(16, 128, 512)
        wq_p = np.concatenate([wq_p[:, :, 0:128], wq_p[:, :, 128:256],
                               wq_p[:, :, 256:384], wq_p[:, :, 384:512]], axis=2)
        wq_p = np.ascontiguousarray(wq_p.transpose(1, 0, 2))  # [128,16,512]
        wkva_p = np.concatenate(
            [wkv_a[:, :512], wkv_a[:, 512::2], wkv_a[:, 513::2]],
            axis=1).astype(f)
        wkva_p = to_pkc(wkva_p, 16)                            # [128,16,576]
        wkbk_p = to_pkc(np.concatenate(
            [wkvb_n[:, 192 * h:192 * h + 64] for h in heads], axis=1), 4)
        wkbv_p = to_pkc(np.concatenate(
            [wkvb_n[:, 192 * h + 64:192 * h + 192] for h in heads], axis=1), 4)
        wout_p = np.ascontiguousarray(np.stack(
            [out_proj_w[128 * h:128 * h + 128, :].astype(f) for h in heads],
            axis=1))                                           # [128,4,D]
        xTb = np.ascontiguousarray(x[b].T).astype(f).reshape(16, 128, S)
        xTb = np.ascontiguousarray(xTb.transpose(1, 0, 2))     # [128,16,S]
        ins.append({
            "xT": xTb,
            "wq": wq_p,
            "wkva": wkva_p,
            "wkbk": wkbk_p,
            "wkbv": wkbv_p,
            "wout": wout_p,
            "cossin": cossin,
            "consts": consts,
        })
    return ins


def kernel(**inputs):
    ins = _prep_inputs(
        inputs["x"], inputs["w_query"], inputs["wkv_a"], inputs["wkv_b"],
        inputs["kv_norm_w"], inputs["out_proj_w"])
    nc = build_nc()
    res = run_bass_kernel_spmd(nc, ins, list(range(8)))
    out = np.zeros((B, S, D), np.float32)
    for c in range(8):
        out[c // 4] += res.results[c]["outp"]
    return out
